# revision 9
# baseline (speedup 1.0000x reference)
"""Causal grouped-query paged attention (prefill) on 8 Trainium2 NeuronCores.

Problem (hardcoded): T=4096 tokens (B=2 seqs x SEQ=2048), 32 q heads,
8 kv heads (GQA group g=4), head_dim=128, paged fp32 KV cache
(512 blocks x 16 tokens).

Sharding: tensor-parallel over KV heads -- core h gets kv head h, its 4
query heads, and both sequences => 8 causal attention slices of
[2048 q x 2048 k x 128 d] per core.

v2 design notes (vs v1 baseline at ~300us):
  - S^T orientation: scores^T[k, q] via K-chunk-stationary matmuls, so the
    P@V matmul needs no transposes.
  - The softmax denominator matmul (ones^T @ P per chunk, 1/3 of all PE
    streaming in v1) is gone: DVE accumulates P chunks elementwise into a
    per-row fp16 acc[128, 512]; the 128-partition reduction and the
    1/denominator normalize both happen host-side on the DMA'd acc.
  - O^T is DMA'd out untransposed and unnormalized (fp32); host transposes.
    This removes all PE transposes and the v1 epilogue dependency chain.
  - exp instructions merged: QK scores for up to 3 k-chunks land in one
    [128, 1536] PSUM tile (3 banks, double-buffered) and get one ScalarE
    activation; ScalarE per-instruction overhead (~190ns) was ~40% of its
    busy time in v1.
  - diagonal (masked) chunks pack gapless into one [128, 1280] extent:
    j0@0 w512 | j1@512 w384 | j3@896 w128 | j2@1024 w256, so one exp and
    three DVE tri-mask multiplies cover the whole diagonal of a q-row.
  - PSUM->SBUF O^T copies stay on DVE (GpSimd has no PSUM port); the
    diagonal tri-mask multiplies go to the otherwise-idle GpSimd.
"""

import math

import numpy as np

import concourse.bass as bass
import concourse.tile as tile
from concourse import bacc, mybir
from concourse.bass_utils import run_bass_kernel_spmd

# problem constants
B = 2
SEQ = 2048
T = B * SEQ
N_QO_HEADS = 32
N_KV_HEADS = 8
G = N_QO_HEADS // N_KV_HEADS  # 4
D = 128
PAGE = 16
NUM_BLOCKS = 512
N_CORES = 8

QTILE = 512   # q chunk (matmul moving dim)
KCH = 128     # k chunk (contraction tile)
GRP = 3       # full k-chunks per PSUM score tile / exp instruction
STW = GRP * QTILE  # 1536 cols = 3 PSUM banks
F32 = mybir.dt.float32
FP16 = mybir.dt.float16
SM_SCALE = 1.0 / math.sqrt(D)

# diagonal packing: (j, st_off, width, qoff) in emission order; gapless
# within [0, 1280) and each chunk's columns stay inside one PSUM bank.
DIAG = [
    (0, 0, 512, 0),
    (1, 512, 384, 128),
    (3, 896, 128, 384),
    (2, 1024, 256, 256),
]
DIAG_EXT = 1280
# tri-mask multiply positions in the packed layout: (st_off, width);
# [896,1152) covers j3's 128 cols plus j2's first 128 via mask2 = tri|tri.
DIAG_MASKS = [(0, 128), (512, 128), (896, 256)]


def emit(nc, n_slices, n_seqs, seq, slice_to_seq):
    """Emit the attention program. Inputs (DRAM):
      qt    [n_slices, 128, seq]  Q^T per slice
      kt    [n_seqs,   128, seq]  K^T per sequence
      v     [n_seqs,   seq, 128]  V per sequence
      mask  [128, 128]            upper-tri 0/1
      mask2 [128, 256]            [tri | tri]
    Outputs:
      ot [n_slices, 128, seq]        O^T, unnormalized, fp32
      ds [n_slices, nq, 128, QTILE]  per-row P partial sums (fp16); host
                                     reduces partitions -> denominators
    """
    nq = seq // QTILE

    qt = nc.dram_tensor("qt", [n_slices, D, seq], FP16, kind="ExternalInput").ap()
    kt = nc.dram_tensor("kt", [n_seqs, D, seq], FP16, kind="ExternalInput").ap()
    v = nc.dram_tensor("v", [n_seqs, seq, D], FP16, kind="ExternalInput").ap()
    mask = nc.dram_tensor("mask", [D, D], FP16, kind="ExternalInput").ap()
    mask2 = nc.dram_tensor("mask2", [D, 256], FP16, kind="ExternalInput").ap()
    ot = nc.dram_tensor("ot", [n_slices, D, seq], F32, kind="ExternalOutput").ap()
    ds = nc.dram_tensor(
        "ds", [n_slices, nq, D, QTILE], FP16, kind="ExternalOutput"
    ).ap()

    with tile.TileContext(nc) as tc:
        with (
            tc.tile_pool(name="const", bufs=1) as const_pool,
            tc.tile_pool(name="kv", bufs=1) as kv_pool,
            tc.tile_pool(name="q", bufs=1) as q_pool,
            tc.tile_pool(name="pt", bufs=3) as pt_pool,
            tc.tile_pool(name="acc", bufs=2) as acc_pool,
            tc.tile_pool(name="osb", bufs=2) as osb_pool,
            tc.tile_pool(name="st", bufs=2, space="PSUM") as st_pool,
            tc.tile_pool(name="ot", bufs=2, space="PSUM") as ot_pool,
        ):
            mask_sb = const_pool.tile([D, D], FP16)
            nc.sync.dma_start(mask_sb[:], mask[:])
            mask2_sb = const_pool.tile([D, 256], FP16)
            nc.sync.dma_start(mask2_sb[:], mask2[:])
            kt_sb = []
            v_sb = []
            for b in range(n_seqs):
                kt_sb.append(kv_pool.tile([D, seq], FP16, tag=f"kt{b}", name=f"ktsb{b}"))
                v_sb.append(kv_pool.tile([D, seq], FP16, tag=f"v{b}", name=f"vsb{b}"))
            qt_sb = [
                q_pool.tile([D, seq], FP16, tag=f"qt{s}", name=f"qtsb{s}")
                for s in range(n_slices)
            ]
            b0 = slice_to_seq[0]
            loaded = set()

            def load_seq(b, split=False):
                if b in loaded:
                    return
                loaded.add(b)
                if split:
                    # halves so slice 0 row 0 can start after ~1/4 the load
                    h = seq // 2
                    nc.sync.dma_start(kt_sb[b][:, :h], kt[b][:, :h])
                    nc.sync.dma_start(kt_sb[b][:, h:], kt[b][:, h:])
                else:
                    nc.sync.dma_start(kt_sb[b][:], kt[b])
                # v chunks packed along free dim: chunk c at cols [c*128, +128)
                nc.sync.dma_start(
                    v_sb[b][:].rearrange("p (c d) -> p c d", d=D),
                    v[b].rearrange("(c p) d -> p c d", p=D),
                )

            load_seq(b0, split=True)
            nc.sync.dma_start(qt_sb[0][:], qt[0])
            for b in range(n_seqs):
                load_seq(b)
            for s in range(1, n_slices):
                nc.sync.dma_start(qt_sb[s][:], qt[s])

            # Build the global group list: per row, the diagonal (masked)
            # group FIRST, then full groups -- the GpSimd mask latency hides
            # behind the row's full-group QK/PV stream.
            # Each entry: (s, qc, kcs, offs, widths, qoffs, masked,
            #              row_first, row_last)
            sched = []
            for s in range(n_slices):
                for qc in range(nq):
                    nfull = (QTILE // KCH) * qc
                    groups = [(
                        [4 * qc + j for j, _, _, _ in DIAG],
                        [off for _, off, _, _ in DIAG],
                        [w for _, _, w, _ in DIAG],
                        [qoff for _, _, _, qoff in DIAG],
                        True,
                    )]
                    for g0 in range(0, nfull, GRP):
                        kcs = list(range(g0, min(g0 + GRP, nfull)))
                        groups.append((
                            kcs,
                            [i * QTILE for i in range(len(kcs))],
                            [QTILE] * len(kcs),
                            [0] * len(kcs),
                            False,
                        ))
                    for gi, g in enumerate(groups):
                        sched.append(
                            (s, qc, *g, gi == 0, gi == len(groups) - 1)
                        )

            # per-row live state, created at row_first, consumed at row_last
            row_state = {}

            def emit_qk_exp(ent):
                s, qc, kcs, offs, widths, qoffs, masked, rf, rl = ent
                b = slice_to_seq[s]
                if rf:
                    row_state[(s, qc)] = (
                        ot_pool.tile([D, QTILE], F32, tag="ot_ps", name="ot_ps"),
                        acc_pool.tile([D, QTILE], FP16, tag="acc", name="acc"),
                    )
                ext = offs[-1] + widths[-1]
                st = st_pool.tile([D, STW], F32, tag="st", name="st")
                for kc, off, w, qoff in zip(kcs, offs, widths, qoffs):
                    nc.tensor.matmul(
                        st[:, off : off + w],
                        lhsT=kt_sb[b][:, kc * KCH : (kc + 1) * KCH],
                        rhs=qt_sb[s][
                            :, qc * QTILE + qoff : qc * QTILE + qoff + w
                        ],
                        start=True,
                        stop=True,
                    )
                pt = pt_pool.tile([D, STW], FP16, tag="pt", name="pt")
                nc.scalar.activation(
                    pt[:, :ext],
                    st[:, :ext],
                    mybir.ActivationFunctionType.Exp,
                    scale=SM_SCALE,
                )
                if masked:
                    # tri masks on GpSimd (SBUF-only engine, else idle)
                    for moff, mw in DIAG_MASKS:
                        msk = mask_sb if mw == 128 else mask2_sb
                        nc.gpsimd.tensor_mul(
                            pt[:, moff : moff + mw],
                            pt[:, moff : moff + mw],
                            msk[:, :mw],
                        )
                return pt

            def emit_pv_ds(ent, pt):
                s, qc, kcs, offs, widths, qoffs, masked, rf, rl = ent
                b = slice_to_seq[s]
                ot_ps, acc = row_state[(s, qc)]
                n = len(kcs)
                for i, (kc, off, w, qoff) in enumerate(
                    zip(kcs, offs, widths, qoffs)
                ):
                    nc.tensor.matmul(
                        ot_ps[:, qoff : qoff + w],
                        lhsT=v_sb[b][:, kc * KCH : (kc + 1) * KCH],
                        rhs=pt[:, off : off + w],
                        start=rf and i == 0,
                        stop=rl and i == n - 1,
                    )
                    if rf and i == 0:
                        # init acc via DMA (SBUF->SBUF) to spare the DVE
                        nc.sync.dma_start(
                            acc[:, qoff : qoff + w], pt[:, off : off + w]
                        )
                    else:
                        nc.vector.tensor_add(
                            acc[:, qoff : qoff + w],
                            acc[:, qoff : qoff + w],
                            pt[:, off : off + w],
                        )
                if rl:
                    # epilogue: O^T out via DVE copy + DMA; acc out raw
                    # (GpSimd has no PSUM port). Returned as a closure so the
                    # caller can defer it behind the next group's DVE adds.
                    def epilogue(s=s, qc=qc, ot_ps=ot_ps, acc=acc):
                        ot_sb = osb_pool.tile([D, QTILE], F32, tag="ot_sb")
                        nc.vector.tensor_copy(ot_sb[:], ot_ps[:])
                        nc.sync.dma_start(
                            ot[s][:, qc * QTILE : (qc + 1) * QTILE], ot_sb[:]
                        )
                        nc.sync.dma_start(ds[s, qc], acc[:])

                    del row_state[(s, qc)]
                    return epilogue
                return None

            # one-group software pipeline: PV/ds of group k emits after
            # QK/exp of group k+1, so the PE never waits on exp/mask latency.
            # Row epilogues are deferred one further group so the DVE queue
            # keeps doing ds adds instead of stalling on the row's last PV.
            pending = None
            pending_epi = None
            for ent in sched:
                pt = emit_qk_exp(ent)
                if pending is not None:
                    epi = emit_pv_ds(*pending)
                    if pending_epi is not None:
                        pending_epi()
                    pending_epi = epi
                pending = (ent, pt)
            epi = emit_pv_ds(*pending)
            if pending_epi is not None:
                pending_epi()
            if epi is not None:
                epi()
    return nc


_CACHE = {}


def _build_full():
    key = "full"
    if key not in _CACHE:
        nc = bacc.Bacc(
            "TRN2",
            target_bir_lowering=False,
            debug=False,
            enable_asserts=False,
            num_devices=N_CORES,
        )
        emit(nc, n_slices=B * G, n_seqs=B, seq=SEQ,
             slice_to_seq=[b for b in range(B) for _ in range(G)])
        nc.compile()
        _CACHE[key] = nc
    return _CACHE[key]


def make_masks():
    tri = np.triu(np.ones((D, D), dtype=np.float16))
    return tri, np.concatenate([tri, tri], axis=1)


def shard_inputs(query, key, value, key_cache, value_cache, block_tables,
                 new_cache_slots):
    """Host-side scatter/gather + head sharding. Returns per-core input maps."""
    kc = key_cache.reshape(NUM_BLOCKS * PAGE, N_KV_HEADS, D).copy()
    vc = value_cache.reshape(NUM_BLOCKS * PAGE, N_KV_HEADS, D).copy()
    kc[new_cache_slots] = key.reshape(T, N_KV_HEADS, D)
    vc[new_cache_slots] = value.reshape(T, N_KV_HEADS, D)
    idx = (
        block_tables[:, :, None].astype(np.int64) * PAGE
        + np.arange(PAGE, dtype=np.int64)[None, None, :]
    ).reshape(B, SEQ)
    k_all = kc[idx]  # [B, SEQ, Hkv, D]
    v_all = vc[idx]
    q_all = query.reshape(B, SEQ, N_KV_HEADS, G, D)
    mask, mask2 = make_masks()

    bf = np.float16
    in_maps = []
    for h in range(N_CORES):
        qt = np.ascontiguousarray(
            q_all[:, :, h, :, :].transpose(0, 2, 3, 1).reshape(B * G, D, SEQ)
        ).astype(bf)
        kt = np.ascontiguousarray(k_all[:, :, h, :].transpose(0, 2, 1)).astype(bf)
        vv = np.ascontiguousarray(v_all[:, :, h, :]).astype(bf)
        in_maps.append({"qt": qt, "kt": kt, "v": vv, "mask": mask, "mask2": mask2})
    return in_maps


def assemble_output(results):
    out = np.empty((B, SEQ, N_KV_HEADS, G, D), dtype=np.float32)
    for h in range(N_CORES):
        o_t = results[h]["ot"]  # [B*G, D, SEQ] f32, unnormalized
        dsv = results[h]["ds"]  # [B*G, nq, D, QTILE] f16
        den = dsv.astype(np.float32).sum(axis=2).reshape(B * G, SEQ)
        o = o_t / den[:, None, :]             # [B*G, D, SEQ]
        oc = o.reshape(B, G, D, SEQ).transpose(0, 3, 1, 2)  # [B, SEQ, G, D]
        out[:, :, h, :, :] = oc
    return out.reshape(T, N_QO_HEADS * D)


def kernel(query, key, value, key_cache, value_cache, block_tables,
           new_cache_slots, _trace=False):
    query = np.asarray(query, dtype=np.float32)
    key = np.asarray(key, dtype=np.float32)
    value = np.asarray(value, dtype=np.float32)
    key_cache = np.asarray(key_cache, dtype=np.float32)
    value_cache = np.asarray(value_cache, dtype=np.float32)
    block_tables = np.asarray(block_tables)
    new_cache_slots = np.asarray(new_cache_slots)

    nc = _build_full()
    in_maps = shard_inputs(query, key, value, key_cache, value_cache,
                           block_tables, new_cache_slots)
    res = run_bass_kernel_spmd(
        nc, in_maps, core_ids=list(range(N_CORES)), trace=_trace
    )
    out = assemble_output(res.results)
    if _trace:
        kernel.last_result = res
    return out


# revision 10
# speedup vs baseline: 1.2434x; 1.2434x over previous
"""Causal grouped-query paged attention (prefill) on 8 Trainium2 NeuronCores.

Problem (hardcoded): T=4096 tokens (B=2 seqs x SEQ=2048), 32 q heads,
8 kv heads (GQA group g=4), head_dim=128, paged fp32 KV cache
(512 blocks x 16 tokens).

Sharding: tensor-parallel over KV heads -- core h gets kv head h, its 4
query heads, and both sequences => 8 causal attention slices of
[2048 q x 2048 k x 128 d] per core.

v2 design notes (vs v1 baseline at ~300us):
  - S^T orientation: scores^T[k, q] via K-chunk-stationary matmuls, so the
    P@V matmul needs no transposes.
  - The softmax denominator matmul (ones^T @ P per chunk, 1/3 of all PE
    streaming in v1) is gone: DVE accumulates P chunks elementwise into a
    per-row fp16 acc[128, 512]; the 128-partition reduction and the
    1/denominator normalize both happen host-side on the DMA'd acc.
  - O^T is DMA'd out untransposed and unnormalized (fp32); host transposes.
    This removes all PE transposes and the v1 epilogue dependency chain.
  - exp instructions merged: QK scores for up to 3 k-chunks land in one
    [128, 1536] PSUM tile (3 banks, double-buffered) and get one ScalarE
    activation; ScalarE per-instruction overhead (~190ns) was ~40% of its
    busy time in v1.
  - diagonal (masked) chunks pack gapless into one [128, 1280] extent:
    j0@0 w512 | j1@512 w384 | j3@896 w128 | j2@1024 w256, so one exp and
    three DVE tri-mask multiplies cover the whole diagonal of a q-row.
  - PSUM->SBUF O^T copies stay on DVE (GpSimd has no PSUM port); the
    diagonal tri-mask multiplies go to the otherwise-idle GpSimd.
"""

import math

import numpy as np

import concourse.bass as bass
import concourse.tile as tile
from concourse import bacc, mybir
from concourse.bass_utils import run_bass_kernel_spmd

# problem constants
B = 2
SEQ = 2048
T = B * SEQ
N_QO_HEADS = 32
N_KV_HEADS = 8
G = N_QO_HEADS // N_KV_HEADS  # 4
D = 128
PAGE = 16
NUM_BLOCKS = 512
N_CORES = 8

QTILE = 512   # q chunk (matmul moving dim)
KCH = 128     # k chunk (contraction tile)
GRP = 3       # full k-chunks per PSUM score tile / exp instruction
STW = GRP * QTILE  # 1536 cols = 3 PSUM banks
F32 = mybir.dt.float32
FP16 = mybir.dt.float16
SM_SCALE = 1.0 / math.sqrt(D)

# diagonal packing: (j, st_off, width, qoff) in emission order; gapless
# within [0, 1280) and each chunk's columns stay inside one PSUM bank.
DIAG = [
    (0, 0, 512, 0),
    (1, 512, 384, 128),
    (3, 896, 128, 384),
    (2, 1024, 256, 256),
]
DIAG_EXT = 1280
# tri-mask multiply positions in the packed layout: (st_off, width);
# [896,1152) covers j3's 128 cols plus j2's first 128 via mask2 = tri|tri.
DIAG_MASKS = [(0, 128), (512, 128), (896, 256)]


def emit(nc, n_slices, n_seqs, seq, slice_to_seq):
    """Emit the attention program. Inputs (DRAM):
      qt    [n_slices, 128, seq]  Q^T per slice
      kt    [n_seqs,   128, seq]  K^T per sequence
      v     [n_seqs,   seq, 128]  V per sequence
      mask  [128, 128]            upper-tri 0/1
      mask2 [128, 256]            [tri | tri]
    Outputs:
      ot [n_slices, 128, seq]        O^T, unnormalized, fp32
      ds [n_slices, nq, 128, QTILE]  per-row P partial sums (fp16); host
                                     reduces partitions -> denominators
    """
    nq = seq // QTILE

    qt = nc.dram_tensor("qt", [n_slices, D, seq], FP16, kind="ExternalInput").ap()
    kt = nc.dram_tensor("kt", [n_seqs, D, seq], FP16, kind="ExternalInput").ap()
    v = nc.dram_tensor("v", [n_seqs, seq, D], FP16, kind="ExternalInput").ap()
    mask = nc.dram_tensor("mask", [D, D], FP16, kind="ExternalInput").ap()
    mask2 = nc.dram_tensor("mask2", [D, 256], FP16, kind="ExternalInput").ap()
    ot = nc.dram_tensor("ot", [n_slices, D, seq], F32, kind="ExternalOutput").ap()
    ds = nc.dram_tensor(
        "ds", [n_slices, nq, D, QTILE], FP16, kind="ExternalOutput"
    ).ap()

    with tile.TileContext(nc) as tc:
        with (
            tc.tile_pool(name="const", bufs=1) as const_pool,
            tc.tile_pool(name="kv", bufs=1) as kv_pool,
            tc.tile_pool(name="q", bufs=1) as q_pool,
            tc.tile_pool(name="pt", bufs=3) as pt_pool,
            tc.tile_pool(name="acc", bufs=2) as acc_pool,
            tc.tile_pool(name="osb", bufs=2) as osb_pool,
            tc.tile_pool(name="st", bufs=2, space="PSUM") as st_pool,
            tc.tile_pool(name="ot", bufs=2, space="PSUM") as ot_pool,
        ):
            mask_sb = const_pool.tile([D, D], FP16)
            nc.sync.dma_start(mask_sb[:], mask[:])
            mask2_sb = const_pool.tile([D, 256], FP16)
            nc.sync.dma_start(mask2_sb[:], mask2[:])
            kt_sb = []
            v_sb = []
            for b in range(n_seqs):
                kt_sb.append(kv_pool.tile([D, seq], FP16, tag=f"kt{b}", name=f"ktsb{b}"))
                v_sb.append(kv_pool.tile([D, seq], FP16, tag=f"v{b}", name=f"vsb{b}"))
            qt_sb = [
                q_pool.tile([D, seq], FP16, tag=f"qt{s}", name=f"qtsb{s}")
                for s in range(n_slices)
            ]
            b0 = slice_to_seq[0]
            loaded = set()

            def load_seq(b, split=False):
                if b in loaded:
                    return
                loaded.add(b)
                if split:
                    # halves so slice 0 row 0 can start after ~1/4 the load
                    h = seq // 2
                    nc.sync.dma_start(kt_sb[b][:, :h], kt[b][:, :h])
                    nc.sync.dma_start(kt_sb[b][:, h:], kt[b][:, h:])
                else:
                    nc.sync.dma_start(kt_sb[b][:], kt[b])
                # v chunks packed along free dim: chunk c at cols [c*128, +128)
                nc.sync.dma_start(
                    v_sb[b][:].rearrange("p (c d) -> p c d", d=D),
                    v[b].rearrange("(c p) d -> p c d", p=D),
                )

            load_seq(b0, split=True)
            nc.sync.dma_start(qt_sb[0][:], qt[0])
            for b in range(n_seqs):
                load_seq(b)
            for s in range(1, n_slices):
                nc.sync.dma_start(qt_sb[s][:], qt[s])

            # Build the global group list: per row, the diagonal (masked)
            # group FIRST, then full groups -- the GpSimd mask latency hides
            # behind the row's full-group QK/PV stream.
            # Each entry: (s, qc, kcs, offs, widths, qoffs, masked,
            #              row_first, row_last)
            sched = []
            for s in range(n_slices):
                for qc in range(nq):
                    nfull = (QTILE // KCH) * qc
                    groups = [(
                        [4 * qc + j for j, _, _, _ in DIAG],
                        [off for _, off, _, _ in DIAG],
                        [w for _, _, w, _ in DIAG],
                        [qoff for _, _, _, qoff in DIAG],
                        True,
                    )]
                    for g0 in range(0, nfull, GRP):
                        kcs = list(range(g0, min(g0 + GRP, nfull)))
                        groups.append((
                            kcs,
                            [i * QTILE for i in range(len(kcs))],
                            [QTILE] * len(kcs),
                            [0] * len(kcs),
                            False,
                        ))
                    for gi, g in enumerate(groups):
                        sched.append(
                            (s, qc, *g, gi == 0, gi == len(groups) - 1)
                        )

            # per-row live state, created at row_first, consumed at row_last
            row_state = {}

            def emit_qk_exp(ent):
                s, qc, kcs, offs, widths, qoffs, masked, rf, rl = ent
                b = slice_to_seq[s]
                if rf:
                    row_state[(s, qc)] = (
                        ot_pool.tile([D, QTILE], F32, tag="ot_ps", name="ot_ps"),
                        acc_pool.tile([D, QTILE], FP16, tag="acc", name="acc"),
                    )
                ext = offs[-1] + widths[-1]
                st = st_pool.tile([D, STW], F32, tag="st", name="st")
                for kc, off, w, qoff in zip(kcs, offs, widths, qoffs):
                    nc.tensor.matmul(
                        st[:, off : off + w],
                        lhsT=kt_sb[b][:, kc * KCH : (kc + 1) * KCH],
                        rhs=qt_sb[s][
                            :, qc * QTILE + qoff : qc * QTILE + qoff + w
                        ],
                        start=True,
                        stop=True,
                    )
                pt = pt_pool.tile([D, STW], FP16, tag="pt", name="pt")
                nc.scalar.activation(
                    pt[:, :ext],
                    st[:, :ext],
                    mybir.ActivationFunctionType.Exp,
                    scale=SM_SCALE,
                )
                if masked:
                    # tri masks on GpSimd (SBUF-only engine, else idle)
                    for moff, mw in DIAG_MASKS:
                        msk = mask_sb if mw == 128 else mask2_sb
                        nc.gpsimd.tensor_mul(
                            pt[:, moff : moff + mw],
                            pt[:, moff : moff + mw],
                            msk[:, :mw],
                        )
                return pt

            def emit_pv_ds(ent, pt):
                s, qc, kcs, offs, widths, qoffs, masked, rf, rl = ent
                b = slice_to_seq[s]
                ot_ps, acc = row_state[(s, qc)]
                n = len(kcs)
                for i, (kc, off, w, qoff) in enumerate(
                    zip(kcs, offs, widths, qoffs)
                ):
                    nc.tensor.matmul(
                        ot_ps[:, qoff : qoff + w],
                        lhsT=v_sb[b][:, kc * KCH : (kc + 1) * KCH],
                        rhs=pt[:, off : off + w],
                        start=rf and i == 0,
                        stop=rl and i == n - 1,
                    )
                    if rf and i == 0:
                        nc.vector.tensor_copy(
                            acc[:, qoff : qoff + w], pt[:, off : off + w]
                        )
                    else:
                        nc.vector.tensor_add(
                            acc[:, qoff : qoff + w],
                            acc[:, qoff : qoff + w],
                            pt[:, off : off + w],
                        )
                if rl:
                    # epilogue: O^T out via DVE copy + DMA; acc out raw
                    # (GpSimd has no PSUM port). Returned as a closure so the
                    # caller can defer it behind the next group's DVE adds.
                    def epilogue(s=s, qc=qc, ot_ps=ot_ps, acc=acc):
                        ot_sb = osb_pool.tile([D, QTILE], F32, tag="ot_sb")
                        nc.vector.tensor_copy(ot_sb[:], ot_ps[:])
                        nc.sync.dma_start(
                            ot[s][:, qc * QTILE : (qc + 1) * QTILE], ot_sb[:]
                        )
                        nc.sync.dma_start(ds[s, qc], acc[:])

                    del row_state[(s, qc)]
                    return epilogue
                return None

            # one-group software pipeline: PV/ds of group k emits after
            # QK/exp of group k+1, so the PE never waits on exp/mask latency.
            # Row epilogues are deferred one further group so the DVE queue
            # keeps doing ds adds instead of stalling on the row's last PV.
            pending = None
            pending_epi = None
            for ent in sched:
                pt = emit_qk_exp(ent)
                if pending is not None:
                    epi = emit_pv_ds(*pending)
                    if pending_epi is not None:
                        pending_epi()
                    pending_epi = epi
                pending = (ent, pt)
            epi = emit_pv_ds(*pending)
            if pending_epi is not None:
                pending_epi()
            if epi is not None:
                epi()
    return nc


_CACHE = {}


def _build_full():
    key = "full"
    if key not in _CACHE:
        nc = bacc.Bacc(
            "TRN2",
            target_bir_lowering=False,
            debug=False,
            enable_asserts=False,
            num_devices=N_CORES,
        )
        emit(nc, n_slices=B * G, n_seqs=B, seq=SEQ,
             slice_to_seq=[b for b in range(B) for _ in range(G)])
        nc.compile()
        _CACHE[key] = nc
    return _CACHE[key]


def make_masks():
    tri = np.triu(np.ones((D, D), dtype=np.float16))
    return tri, np.concatenate([tri, tri], axis=1)


def shard_inputs(query, key, value, key_cache, value_cache, block_tables,
                 new_cache_slots):
    """Host-side scatter/gather + head sharding. Returns per-core input maps."""
    kc = key_cache.reshape(NUM_BLOCKS * PAGE, N_KV_HEADS, D).copy()
    vc = value_cache.reshape(NUM_BLOCKS * PAGE, N_KV_HEADS, D).copy()
    kc[new_cache_slots] = key.reshape(T, N_KV_HEADS, D)
    vc[new_cache_slots] = value.reshape(T, N_KV_HEADS, D)
    idx = (
        block_tables[:, :, None].astype(np.int64) * PAGE
        + np.arange(PAGE, dtype=np.int64)[None, None, :]
    ).reshape(B, SEQ)
    k_all = kc[idx]  # [B, SEQ, Hkv, D]
    v_all = vc[idx]
    q_all = query.reshape(B, SEQ, N_KV_HEADS, G, D)
    mask, mask2 = make_masks()

    bf = np.float16
    in_maps = []
    for h in range(N_CORES):
        qt = np.ascontiguousarray(
            q_all[:, :, h, :, :].transpose(0, 2, 3, 1).reshape(B * G, D, SEQ)
        ).astype(bf)
        kt = np.ascontiguousarray(k_all[:, :, h, :].transpose(0, 2, 1)).astype(bf)
        vv = np.ascontiguousarray(v_all[:, :, h, :]).astype(bf)
        in_maps.append({"qt": qt, "kt": kt, "v": vv, "mask": mask, "mask2": mask2})
    return in_maps


def assemble_output(results):
    out = np.empty((B, SEQ, N_KV_HEADS, G, D), dtype=np.float32)
    for h in range(N_CORES):
        o_t = results[h]["ot"]  # [B*G, D, SEQ] f32, unnormalized
        dsv = results[h]["ds"]  # [B*G, nq, D, QTILE] f16
        den = dsv.astype(np.float32).sum(axis=2).reshape(B * G, SEQ)
        o = o_t / den[:, None, :]             # [B*G, D, SEQ]
        oc = o.reshape(B, G, D, SEQ).transpose(0, 3, 1, 2)  # [B, SEQ, G, D]
        out[:, :, h, :, :] = oc
    return out.reshape(T, N_QO_HEADS * D)


def kernel(query, key, value, key_cache, value_cache, block_tables,
           new_cache_slots, _trace=False):
    query = np.asarray(query, dtype=np.float32)
    key = np.asarray(key, dtype=np.float32)
    value = np.asarray(value, dtype=np.float32)
    key_cache = np.asarray(key_cache, dtype=np.float32)
    value_cache = np.asarray(value_cache, dtype=np.float32)
    block_tables = np.asarray(block_tables)
    new_cache_slots = np.asarray(new_cache_slots)

    nc = _build_full()
    in_maps = shard_inputs(query, key, value, key_cache, value_cache,
                           block_tables, new_cache_slots)
    res = run_bass_kernel_spmd(
        nc, in_maps, core_ids=list(range(N_CORES)), trace=_trace
    )
    out = assemble_output(res.results)
    if _trace:
        kernel.last_result = res
    return out
